# revision 5
# baseline (speedup 1.0000x reference)
"""ARMANet (2-layer ARMAConv K=1,T=1 + mean-pool + fc) on 8 TRN2 NeuronCores.

v2: restructured to kill per-instruction overheads found in the v1 trace:
 - gather calls are large multi-window chunks (q-major slot layout), cutting
   SWDGE descriptor-gen calls from 784 to ~80
 - one-hot matrices built 16 tiles per DVE instruction via broadcast APs
 - phase 0 (g1 = (dinv*x) @ W1i) computed replicated on every core, removing
   the first AllGather entirely
 - assembly (root/relu/scale) done as fused full-slice ops, not per-window
 - segment sums accumulate in SBUF across source quarters

Self-contained: hardcodes shapes for N=100000, E=1600000, IN=HID=64,
OUT=32, NUM_GRAPHS=64.
"""
import os
import sys

sys.path.insert(0, '/opt/trn_rl_repo')

import numpy as np

import concourse.bass as bass
import concourse.tile as tile
from concourse import bacc, mybir
from concourse.bass_utils import run_bass_kernel_spmd

N = 100000
E = 1600000
IN_DIM = 64
HID = 64
OUT = 32
NUM_GRAPHS = 64

P = 128
NCORES = 8
SLICE = 12544              # nodes per core (padded)
WPC = SLICE // P           # 98 windows per core
NPAD = SLICE * NCORES      # 100352
NQ = 4                     # src quarters (int16 index limit)
QSIZE = NPAD // NQ         # 25088
CHT = 48                   # tiles per gather call
OHB = 16                   # one-hot tiles built per DVE instruction
SENTINEL = 600.0           # one-hot miss value for padded slots

F32 = mybir.dt.float32
F16 = mybir.dt.float16
I16 = mybir.dt.int16

_cache = {}


def _install_trace_hook():
    """Register the NTFF profile hook so trace=True works under axon."""
    import types
    if 'antenv.axon_hooks' in sys.modules:
        return
    mod = types.ModuleType("antenv.axon_hooks")
    mod._hook = None
    mod.set_axon_ntff_profile_hook = lambda h: setattr(mod, '_hook', h)
    mod.get_axon_ntff_profile_hook = lambda: mod._hook
    sys.modules['antenv.axon_hooks'] = mod
    import antenv
    antenv.axon_hooks = mod
    try:
        from trn_agent_boot.trn_boot import _ntff_profile_via_ctypes
        mod.set_axon_ntff_profile_hook(
            _ntff_profile_via_ctypes('/opt/axon/libaxon_pjrt.so'))
    except Exception:
        pass


def _prep(x, edge_index, batch, dinv):
    """Host-side shard prep. Returns per-core arrays + static schedule."""
    row = np.asarray(edge_index[0], dtype=np.int64)
    col = np.asarray(edge_index[1], dtype=np.int64)

    c_of = col // SLICE
    w_of = (col % SLICE) // P
    d_rel = (col % P).astype(np.float16)
    q_of = row // QSIZE
    src_rel = (row % QSIZE).astype(np.int16)

    # group (q, w) per core; SPMD-uniform tile counts = max over cores
    key_cqw = (c_of * NQ + q_of) * WPC + w_of
    counts = np.bincount(key_cqw, minlength=NCORES * NQ * WPC)
    counts = counts.reshape(NCORES, NQ, WPC)
    gmax = counts.max(axis=0)                  # [NQ, WPC]
    T = -(-gmax // P)                          # tiles per (q, w)

    tiles_flat = T.reshape(-1)                 # q-major
    tile_base = np.zeros(NQ * WPC, np.int64)
    np.cumsum(tiles_flat[:-1], out=tile_base[1:])
    n_tiles = int(tiles_flat.sum())
    n_slots = n_tiles * P

    # per-quarter window runs and gather-call chunks
    qsched = []   # per q: dict(calls=[(t0, nt)], wins=[(w, t0, nt)])
    for q in range(NQ):
        qt0 = int(tile_base[q * WPC])
        qt1 = qt0 + int(T[q].sum())
        calls = []
        t = qt0
        while t < qt1:
            nt = min(CHT, qt1 - t)
            calls.append((t, nt))
            t += nt
        wins = []
        for w in range(WPC):
            if T[q, w] > 0:
                wins.append((w, int(tile_base[q * WPC + w]), int(T[q, w])))
        qsched.append(dict(calls=calls, wins=wins, qt0=qt0, qt1=qt1))

    first_q = np.full(WPC, -1, np.int64)
    for q in range(NQ - 1, -1, -1):
        first_q[T[q] > 0] = q

    # per-edge slot assignment (sorted by core, quarter, window, src)
    order = np.lexsort((row, w_of, q_of, c_of))
    k = key_cqw[order]
    change = np.empty(E, bool)
    change[0] = True
    change[1:] = k[1:] != k[:-1]
    starts = np.flatnonzero(change)
    grp = np.cumsum(change) - 1
    rank = np.arange(E) - starts[grp]
    qw = (q_of * WPC + w_of)[order]
    slot = tile_base[qw] * P + rank

    srcrel_slots = np.zeros((NCORES, n_slots), np.int16)
    dcol_slots = np.full((NCORES, n_slots), SENTINEL, np.float16)
    cs = c_of[order]
    srcrel_slots[cs, slot] = src_rel[order]
    dcol_slots[cs, slot] = d_rel[order]

    # idx int16 wrapped layout: [16, n_slots/16] tiled to 128 partitions
    idx16 = np.empty((NCORES, P, n_slots // 16), np.int16)
    for c in range(NCORES):
        wrap = srcrel_slots[c].reshape(n_slots // 16, 16).T
        idx16[c] = np.tile(wrap, (8, 1))

    dcol = np.empty((NCORES, P, n_tiles), np.float16)
    for c in range(NCORES):
        dcol[c] = dcol_slots[c].reshape(n_tiles, P).T

    # node-side arrays
    xpad = np.zeros((NPAD, IN_DIM), np.float32)
    xpad[:N] = x
    dinv_pad = np.zeros(NPAD, np.float32)
    dinv_pad[:N] = dinv
    batch_pad = np.full(NPAD, SENTINEL, np.float32)
    batch_pad[:N] = np.asarray(batch, np.float32)

    xdT = (xpad * dinv_pad[:, None]).T.astype(np.float16)   # [64, NPAD]
    xTr = np.empty((NCORES, IN_DIM, SLICE), np.float16)
    dinv_cols = np.empty((NCORES, P, WPC), np.float32)
    batch_cols = np.empty((NCORES, P, WPC), np.float16)
    for c in range(NCORES):
        sl = slice(c * SLICE, (c + 1) * SLICE)
        xTr[c] = xpad[sl].T
        dinv_cols[c] = dinv_pad[sl].reshape(WPC, P).T
        batch_cols[c] = batch_pad[sl].reshape(WPC, P).T.astype(np.float16)

    sched = dict(n_slots=n_slots, n_tiles=n_tiles, qsched=qsched,
                 first_q=first_q.tolist())
    data = dict(idx16=idx16, dcol=dcol, xdT=xdT, xTr=xTr,
                dinv_cols=dinv_cols, batch_cols=batch_cols)
    return sched, data


def _build(sched):
    """Build the SPMD Bass program (same for all cores)."""
    from contextlib import ExitStack

    n_tiles = sched['n_tiles']
    n_slots = sched['n_slots']
    qsched = sched['qsched']
    first_q = sched['first_q']

    nc = bacc.Bacc("TRN2", target_bir_lowering=False, debug=False,
                   num_devices=NCORES, num_swdge_queues=4)

    # I/O
    xdT_d = nc.dram_tensor("xdT", [IN_DIM, NPAD], F16, kind="ExternalInput")
    xTr_d = nc.dram_tensor("xTr", [IN_DIM, SLICE], F16, kind="ExternalInput")
    dinv_d = nc.dram_tensor("dinv_cols", [P, WPC], F32, kind="ExternalInput")
    batch_d = nc.dram_tensor("batch_cols", [P, WPC], F16, kind="ExternalInput")
    idx_d = nc.dram_tensor("idx16", [P, n_slots // 16], I16, kind="ExternalInput")
    dcol_d = nc.dram_tensor("dcol", [P, n_tiles], F16, kind="ExternalInput")
    w1i_d = nc.dram_tensor("w1i", [IN_DIM, HID], F16, kind="ExternalInput")
    w1r_d = nc.dram_tensor("w1r", [IN_DIM, HID], F16, kind="ExternalInput")
    b1r_d = nc.dram_tensor("b1r", [P, HID], F32, kind="ExternalInput")
    w2i_d = nc.dram_tensor("w2i", [HID, HID], F32, kind="ExternalInput")
    w2r_d = nc.dram_tensor("w2r", [HID, HID], F32, kind="ExternalInput")
    b2r_d = nc.dram_tensor("b2r", [P, HID], F32, kind="ExternalInput")
    fcwb_d = nc.dram_tensor("fcwb", [HID + 1, OUT], F32, kind="ExternalInput")
    cntinv_d = nc.dram_tensor("cntinv", [NUM_GRAPHS, 1], F32, kind="ExternalInput")
    out_d = nc.dram_tensor("out", [NUM_GRAPHS, OUT], F32, kind="ExternalOutput")

    # constants embedded in NEFF
    iota_oh_np = np.tile(np.arange(P, dtype=np.float16)[None, :], (P, OHB))
    iota_oh_d = nc.inline_tensor(iota_oh_np, name="iota_oh")
    iota_pool_np = np.tile(np.arange(NUM_GRAPHS, dtype=np.float16)[None, :], (P, 1))
    iota_pool_d = nc.inline_tensor(iota_pool_np, name="iota_pool")
    ident_d = nc.inline_tensor(np.eye(P, dtype=np.float32), name="ident128")

    # internal DRAM
    g1tab = nc.dram_tensor("g1tab", [NPAD, P], F16)
    g2loc = nc.dram_tensor("g2loc", [SLICE, P], F16)
    g2tab = nc.dram_tensor("g2tab", [NPAD, P], F16, addr_space="Shared")
    pin_d = nc.dram_tensor("pooled_in", [NUM_GRAPHS, HID], F32)
    pout_d = nc.dram_tensor("pooled_out", [NUM_GRAPHS, HID], F32,
                            addr_space="Shared")

    RG = [list(range(NCORES))]
    Relu = mybir.ActivationFunctionType.Relu
    Copy = mybir.ActivationFunctionType.Copy

    with tile.TileContext(nc) as tc:
        with ExitStack() as ctx:
            consts = ctx.enter_context(tc.tile_pool(name="consts", bufs=1))
            canvas = ctx.enter_context(tc.tile_pool(name="canvas", bufs=4))
            gpool = ctx.enter_context(tc.tile_pool(name="gbuf", bufs=2))
            ohpool = ctx.enter_context(tc.tile_pool(name="oh", bufs=4))
            wk = ctx.enter_context(tc.tile_pool(name="wk", bufs=4))
            pseg = ctx.enter_context(tc.tile_pool(name="pseg", bufs=3, space="PSUM"))
            pbat = ctx.enter_context(tc.tile_pool(name="pbat", bufs=2, space="PSUM"))
            ptp = ctx.enter_context(tc.tile_pool(name="ptp", bufs=2, space="PSUM"))
            ppool = ctx.enter_context(tc.tile_pool(name="ppool", bufs=1, space="PSUM"))

            # ---- load constants ----
            iota_oh_sb = consts.tile([P, OHB * P], F16)
            nc.sync.dma_start(out=iota_oh_sb[:], in_=iota_oh_d[:, :])
            iota_pool_sb = consts.tile([P, NUM_GRAPHS], F16)
            nc.sync.dma_start(out=iota_pool_sb[:], in_=iota_pool_d[:, :])
            ident_sb = consts.tile([P, P], F32)
            nc.sync.dma_start(out=ident_sb[:], in_=ident_d[:, :])
            dinv_sb = consts.tile([P, WPC], F32)
            nc.sync.dma_start(out=dinv_sb[:], in_=dinv_d[:, :])
            batch_sb = consts.tile([P, WPC], F16)
            nc.sync.dma_start(out=batch_sb[:], in_=batch_d[:, :])
            idx_sb = consts.tile([P, n_slots // 16], I16)
            nc.sync.dma_start(out=idx_sb[:], in_=idx_d[:, :])
            dcol_sb = consts.tile([P, n_tiles], F16)
            nc.sync.dma_start(out=dcol_sb[:], in_=dcol_d[:, :])
            w1i_sb = consts.tile([IN_DIM, HID], F16)
            nc.sync.dma_start(out=w1i_sb[:], in_=w1i_d[:, :])
            w1r_sb = consts.tile([IN_DIM, HID], F16)
            nc.sync.dma_start(out=w1r_sb[:], in_=w1r_d[:, :])
            b1r_sb = consts.tile([P, HID], F32)
            nc.sync.dma_start(out=b1r_sb[:], in_=b1r_d[:, :])
            w2i_sb = consts.tile([HID, HID], F32)
            nc.sync.dma_start(out=w2i_sb[:], in_=w2i_d[:, :])
            w2r_sb = consts.tile([HID, HID], F32)
            nc.sync.dma_start(out=w2r_sb[:], in_=w2r_d[:, :])
            b2r_sb = consts.tile([P, HID], F32)
            nc.sync.dma_start(out=b2r_sb[:], in_=b2r_d[:, :])
            fcwb_sb = consts.tile([HID + 1, OUT], F32)
            nc.sync.dma_start(out=fcwb_sb[:], in_=fcwb_d[:, :])
            cntinv_sb = consts.tile([NUM_GRAPHS, 1], F32)
            nc.sync.dma_start(out=cntinv_sb[:], in_=cntinv_d[:, :])

            CSLOT_F32 = [P, WPC * HID]      # 25088B canvas slot, f32 view
            CSLOT_F16 = [P, WPC * P]        # same bytes, f16 view
            XSLOT = [IN_DIM, SLICE]         # same bytes, f16 on 64 partitions

            def canvas_tile(shape, dtype, name):
                return canvas.tile(shape, dtype, tag="big", name=name)

            # ---- phase 0: g1 table, replicated on every core ----
            for c in range(NCORES):
                xTc = canvas_tile(XSLOT, F16, f"xTc{c}")
                nc.sync.dma_start(out=xTc[:],
                                  in_=xdT_d[:, c * SLICE:(c + 1) * SLICE])
                stg = canvas_tile(CSLOT_F16, F16, f"stg{c}")
                stg3 = stg[:].rearrange("p (w d) -> p w d", d=P)
                for b in range(0, WPC, 8):
                    nb = min(8, WPC - b)
                    bat = pbat.tile([P, 8 * HID], F32, tag="bat", name="p0bat")
                    bat3 = bat[:].rearrange("p (k d) -> p k d", d=HID)
                    for k in range(nb):
                        w = b + k
                        nc.tensor.matmul(
                            bat3[:, k, :],
                            lhsT=xTc[:, w * P:(w + 1) * P],
                            rhs=w1i_sb[:], start=True, stop=True)
                    nc.scalar.activation(stg3[:, b:b + nb, 0:HID],
                                         bat3[:, 0:nb, :], Copy)
                nc.sync.dma_start(
                    out=g1tab[c * SLICE:(c + 1) * SLICE, :]
                        .rearrange("(w p) d -> p w d", p=P),
                    in_=stg3)

            def propagate(gsrc, accum, tag):
                """Chunked gather + fused one-hot + per-window PSUM->SBUF."""
                accum3 = accum[:].rearrange("p (w d) -> p w d", d=HID)
                callno = 0
                for q in range(NQ):
                    qs = qsched[q]
                    # window lookup by tile
                    win_of = {}
                    for (w, t0, nt) in qs['wins']:
                        for t in range(t0, t0 + nt):
                            win_of[t] = (w, t == t0, t == t0 + nt - 1)
                    for (t0, nt) in qs['calls']:
                        gbuf = gpool.tile([P, CHT * P], F16, tag="gb",
                                          name="gbuf")
                        gb3 = gbuf[:].rearrange("p (s d) -> p s d", d=P)
                        nc.gpsimd.dma_gather(
                            out_ap=gb3[:, 0:nt, :],
                            in_ap=gsrc[q * QSIZE:(q + 1) * QSIZE, :],
                            idxs_ap=idx_sb[:, t0 * 8:(t0 + nt) * 8],
                            num_idxs=nt * P, num_idxs_reg=nt * P, elem_size=P,
                            single_packet=False, queue_num=callno % 4)
                        callno += 1
                        # fused one-hot blocks
                        ohs = {}
                        for ob in range(0, nt, OHB):
                            nb = min(OHB, nt - ob)
                            oh = ohpool.tile([P, OHB * P], F16, tag="oh",
                                             name="oh")
                            oh3 = oh[:].rearrange("p (k d) -> p k d", d=P)
                            nc.vector.tensor_tensor(
                                oh3[:, 0:nb, :],
                                iota_oh_sb[:].rearrange(
                                    "p (k d) -> p k d", d=P)[:, 0:nb, :],
                                dcol_sb[:, t0 + ob:t0 + ob + nb]
                                    .unsqueeze(2).to_broadcast([P, nb, P]),
                                mybir.AluOpType.is_equal)
                            ohs[ob] = oh3
                        for tt in range(nt):
                            t = t0 + tt
                            w, fst, lst = win_of[t]
                            if fst:
                                seg = pseg.tile([P, HID], F32, tag="seg",
                                                name="seg")
                            nc.tensor.matmul(
                                seg[:], lhsT=ohs[(tt // OHB) * OHB][:, tt % OHB, :],
                                rhs=gb3[:, tt, 0:HID], start=fst, stop=lst)
                            if lst:
                                if first_q[w] == q:
                                    nc.vector.tensor_copy(accum3[:, w, :], seg[:])
                                else:
                                    nc.vector.tensor_add(
                                        accum3[:, w, :], accum3[:, w, :], seg[:])

            def root_build(rootbuf, xT_sb, w_sb, br_sb):
                """rootbuf[:, w*64:(w+1)*64] = xT_w @ w + b (batched by 8)."""
                r3 = rootbuf[:].rearrange("p (w d) -> p w d", d=HID)
                for b in range(0, WPC, 8):
                    nb = min(8, WPC - b)
                    bat = pbat.tile([P, 8 * HID], F32, tag="bat", name="rbat")
                    bat3 = bat[:].rearrange("p (k d) -> p k d", d=HID)
                    for k in range(nb):
                        w = b + k
                        nc.tensor.matmul(
                            bat3[:, k, :],
                            lhsT=xT_sb[:, w * P:(w + 1) * P],
                            rhs=w_sb[:], start=True, stop=True)
                    nc.vector.tensor_add(
                        r3[:, b:b + nb, :], bat3[:, 0:nb, :],
                        br_sb[:].unsqueeze(1).to_broadcast([P, nb, HID]))

            def fuse_h(hbuf, accum, rootbuf):
                """h = relu(accum * dinv_dest + root), full-slice fused."""
                h3 = hbuf[:].rearrange("p (w d) -> p w d", d=HID)
                a3 = accum[:].rearrange("p (w d) -> p w d", d=HID)
                nc.vector.tensor_tensor(
                    h3, a3,
                    dinv_sb[:].unsqueeze(2).to_broadcast([P, WPC, HID]),
                    mybir.AluOpType.mult)
                nc.vector.tensor_add(hbuf[:], hbuf[:], rootbuf[:])
                nc.vector.tensor_scalar_max(hbuf[:], hbuf[:], 0.0)

            # ---- layer 1 ----
            accum1 = canvas_tile(CSLOT_F32, F32, "accum1")
            propagate(g1tab, accum1, "a")
            xTrS = canvas_tile(XSLOT, F16, "xTrS")
            nc.sync.dma_start(out=xTrS[:], in_=xTr_d[:, :])
            root1 = canvas_tile(CSLOT_F32, F32, "root1")
            root_build(root1, xTrS, w1r_sb, b1r_sb)
            h1 = canvas_tile(CSLOT_F32, F32, "h1")
            fuse_h(h1, accum1, root1)
            h1s = canvas_tile(CSLOT_F32, F32, "h1s")
            h1s3 = h1s[:].rearrange("p (w d) -> p w d", d=HID)
            nc.vector.tensor_tensor(
                h1s3, h1[:].rearrange("p (w d) -> p w d", d=HID),
                dinv_sb[:].unsqueeze(2).to_broadcast([P, WPC, HID]),
                mybir.AluOpType.mult)

            # per-4-window transposes; per-8-window g2/root2 matmuls
            stage1 = canvas_tile(CSLOT_F16, F16, "stage1")
            st3 = stage1[:].rearrange("p (w d) -> p w d", d=P)
            root2 = canvas_tile(CSLOT_F32, F32, "root2")
            r23 = root2[:].rearrange("p (w d) -> p w d", d=HID)
            h13 = h1[:].rearrange("p (w d) -> p w d", d=HID)

            for b4 in range(0, WPC, 4):
                n4 = min(4, WPC - b4)
                tpA = ptp.tile([IN_DIM, 4 * P], F32, tag="tp", name="tpA")
                tpB = ptp.tile([IN_DIM, 4 * P], F32, tag="tp", name="tpB")
                for k in range(n4):
                    w = b4 + k
                    nc.tensor.transpose(tpA[:, k * P:(k + 1) * P],
                                        h13[:, w, :], ident_sb[:])
                    nc.tensor.transpose(tpB[:, k * P:(k + 1) * P],
                                        h1s3[:, w, :], ident_sb[:])
                h1T4 = wk.tile([IN_DIM, 4 * P], F32, tag="h1T", bufs=3,
                               name="h1T4")
                nc.scalar.activation(h1T4[:], tpA[:], Copy)
                h1sT4 = wk.tile([IN_DIM, 4 * P], F32, tag="h1sT", bufs=3,
                                name="h1sT4")
                nc.scalar.activation(h1sT4[:], tpB[:], Copy)
                batG = pbat.tile([P, 8 * HID], F32, tag="bat", name="batG")
                batG3 = batG[:].rearrange("p (k d) -> p k d", d=HID)
                batR = pbat.tile([P, 8 * HID], F32, tag="bat", name="batR")
                batR3 = batR[:].rearrange("p (k d) -> p k d", d=HID)
                for k in range(n4):
                    nc.tensor.matmul(batG3[:, k, :],
                                     lhsT=h1sT4[:, k * P:(k + 1) * P],
                                     rhs=w2i_sb[:], start=True, stop=True)
                    nc.tensor.matmul(batR3[:, k, :],
                                     lhsT=h1T4[:, k * P:(k + 1) * P],
                                     rhs=w2r_sb[:], start=True, stop=True)
                nc.scalar.activation(st3[:, b4:b4 + n4, 0:HID],
                                     batG3[:, 0:n4, :], Copy)
                nc.vector.tensor_add(
                    r23[:, b4:b4 + n4, :], batR3[:, 0:n4, :],
                    b2r_sb[:].unsqueeze(1).to_broadcast([P, n4, HID]))

            nc.sync.dma_start(
                out=g2loc.ap().rearrange("(w p) d -> p w d", p=P), in_=st3)
            nc.gpsimd.collective_compute(
                "AllGather", mybir.AluOpType.bypass, replica_groups=RG,
                ins=[g2loc.ap().opt()], outs=[g2tab.ap().opt()])

            # ---- layer 2 ----
            accum2 = canvas_tile(CSLOT_F32, F32, "accum2")
            propagate(g2tab, accum2, "b")
            h2 = canvas_tile(CSLOT_F32, F32, "h2")
            fuse_h(h2, accum2, root2)

            # pooling: one fused one-hot over graphs, then 98 matmuls
            ohB = canvas_tile(CSLOT_F32, F32, "ohB")
            ohB3 = ohB[:].rearrange("p (w d) -> p w d", d=HID)
            nc.vector.tensor_tensor(
                ohB3,
                iota_pool_sb[:].unsqueeze(1).to_broadcast([P, WPC, NUM_GRAPHS]),
                batch_sb[:].unsqueeze(2).to_broadcast([P, WPC, NUM_GRAPHS]),
                mybir.AluOpType.is_equal)
            h23 = h2[:].rearrange("p (w d) -> p w d", d=HID)
            pool_ps = ppool.tile([NUM_GRAPHS, HID], F32)
            for w in range(WPC):
                nc.tensor.matmul(pool_ps[:], lhsT=ohB3[:, w, :],
                                 rhs=h23[:, w, :],
                                 start=(w == 0), stop=(w == WPC - 1))

            pooled_sb = wk.tile([NUM_GRAPHS, HID], F32, tag="fin", name="pooled")
            nc.vector.tensor_copy(pooled_sb[:], pool_ps[:])
            nc.sync.dma_start(out=pin_d[:, :], in_=pooled_sb[:])
            nc.gpsimd.collective_compute(
                "AllReduce", mybir.AluOpType.add, replica_groups=RG,
                ins=[pin_d.ap().opt()], outs=[pout_d.ap().opt()])
            pg = wk.tile([NUM_GRAPHS, HID], F32, tag="fin", name="pg")
            nc.sync.dma_start(out=pg[:], in_=pout_d[:, :])
            pm = wk.tile([NUM_GRAPHS, HID], F32, tag="fin", name="pm")
            nc.vector.tensor_scalar_mul(pm[:], pg[:], cntinv_sb[:])
            tpf = ptp.tile([IN_DIM, 4 * P], F32, tag="tp", name="tpf")
            nc.tensor.transpose(tpf[0:HID, 0:NUM_GRAPHS], pm[:],
                                ident_sb[0:NUM_GRAPHS, 0:NUM_GRAPHS])
            lhs_fc = wk.tile([HID + 1, NUM_GRAPHS], F32, tag="fin", name="lhsfc")
            nc.vector.tensor_copy(lhs_fc[0:HID, :], tpf[0:HID, 0:NUM_GRAPHS])
            nc.vector.memset(lhs_fc[HID:HID + 1, :], 1.0)
            out_ps = pbat.tile([P, 8 * HID], F32, tag="bat", name="outps")
            nc.tensor.matmul(out_ps[0:NUM_GRAPHS, 0:OUT], lhsT=lhs_fc[:],
                             rhs=fcwb_sb[:], start=True, stop=True)
            osb = wk.tile([NUM_GRAPHS, OUT], F32, tag="fin", name="osb")
            nc.vector.tensor_copy(osb[:], out_ps[0:NUM_GRAPHS, 0:OUT])
            nc.sync.dma_start(out=out_d[:, :], in_=osb[:])

    nc.compile()
    return nc


def kernel(x, edge_index, batch, w1_init, w1_root, b1, w2_init, w2_root, b2,
           fc_w, fc_b):
    x = np.asarray(x, np.float32)
    edge_index = np.asarray(edge_index)
    batch = np.asarray(batch)

    col = edge_index[1].astype(np.int64)
    deg = np.bincount(col, minlength=N).astype(np.float32)
    dinv = np.where(deg > 0, 1.0 / np.sqrt(np.maximum(deg, 1.0)), 0.0).astype(np.float32)

    sched, data = _prep(x, edge_index, batch, dinv)

    key = (sched['n_slots'], sched['n_tiles'])
    if key not in _cache:
        _cache[key] = _build(sched)
    nc = _cache[key]

    cnt = np.bincount(batch.astype(np.int64), minlength=NUM_GRAPHS).astype(np.float32)
    cntinv = (1.0 / np.maximum(cnt, 1.0)).reshape(NUM_GRAPHS, 1)
    fcwb = np.vstack([np.asarray(fc_w, np.float32),
                      np.asarray(fc_b, np.float32)[None, :]])
    b1r = np.tile(np.asarray(b1, np.float32)[None, :], (P, 1))
    b2r = np.tile(np.asarray(b2, np.float32)[None, :], (P, 1))

    in_maps = []
    for c in range(NCORES):
        in_maps.append({
            "xdT": data['xdT'],
            "xTr": data['xTr'][c],
            "dinv_cols": data['dinv_cols'][c],
            "batch_cols": data['batch_cols'][c],
            "idx16": data['idx16'][c],
            "dcol": data['dcol'][c],
            "w1i": np.asarray(w1_init, np.float16),
            "w1r": np.asarray(w1_root, np.float16),
            "b1r": b1r,
            "w2i": np.asarray(w2_init, np.float32),
            "w2r": np.asarray(w2_root, np.float32),
            "b2r": b2r,
            "fcwb": fcwb,
            "cntinv": cntinv,
        })

    trace = os.environ.get("GNN_TRACE", "0") == "1"
    kw = {}
    if trace:
        _install_trace_hook()
        kw = dict(trace=True, tmpdir=os.environ.get("GNN_TRACEDIR") or None)
    res = run_bass_kernel_spmd(nc, in_maps, core_ids=list(range(NCORES)), **kw)
    kernel.last_exec_time_ns = res.exec_time_ns
    return np.asarray(res.results[0]["out"], np.float32)


# revision 16
# speedup vs baseline: 1.4361x; 1.4361x over previous
"""ARMANet (2-layer ARMAConv K=1,T=1 + mean-pool + fc) on 8 TRN2 NeuronCores.

v3: the gather descriptor generation on the Q7 (≈3ns/idx, ≈720us/layer) is
the hard bottleneck, so the schedule keeps the Q7 busy continuously:
 - each layer's propagate is split into two destination-halves; assembly of
   half A (and its chunk of the AllGather) overlaps half B's gathers
 - the g-tables use a permuted row order (dest-half, core, window) so the
   two AllGather chunks land exactly on int16 index quarters
 - g1 is computed replicated (no first AllGather); its 4 quarter tensors
   give fine-grained dependencies so layer-1 gathers start early
 - gather calls are 16 tiles (2048 idxs) on rotating SWDGE queues; one
   fused one-hot build per call; fused full-slice assembly ops

Self-contained: hardcodes shapes for N=100000, E=1600000, IN=HID=64,
OUT=32, NUM_GRAPHS=64.
"""
import os
import sys

sys.path.insert(0, '/opt/trn_rl_repo')

import numpy as np

import concourse.bass as bass
import concourse.tile as tile
from concourse import bacc, mybir
from concourse.bass_utils import run_bass_kernel_spmd

N = 100000
E = 1600000
IN_DIM = 64
HID = 64
OUT = 32
NUM_GRAPHS = 64

P = 128
NCORES = 8
SLICE = 12544              # nodes per core (padded)
WPC = SLICE // P           # 98 windows per core
WH = 49                    # windows per dest-half
HROWS = WH * P             # 6272 rows per (core, half)
NPAD = SLICE * NCORES      # 100352
NQ = 4                     # src quarters (int16 index limit)
QSIZE = NPAD // NQ         # 25088
HALFTAB = 2 * QSIZE        # 50176 rows per AllGather chunk
CHT = 12                   # tiles per gather call (= one one-hot block)
SENTINEL = 600.0           # one-hot miss value for padded slots

F32 = mybir.dt.float32
F16 = mybir.dt.float16
I16 = mybir.dt.int16

_cache = {}


def _install_trace_hook():
    """Register the NTFF profile hook so trace=True works under axon."""
    import types
    if 'antenv.axon_hooks' in sys.modules:
        return
    mod = types.ModuleType("antenv.axon_hooks")
    mod._hook = None
    mod.set_axon_ntff_profile_hook = lambda h: setattr(mod, '_hook', h)
    mod.get_axon_ntff_profile_hook = lambda: mod._hook
    sys.modules['antenv.axon_hooks'] = mod
    import antenv
    antenv.axon_hooks = mod
    try:
        from trn_agent_boot.trn_boot import _ntff_profile_via_ctypes
        mod.set_axon_ntff_profile_hook(
            _ntff_profile_via_ctypes('/opt/axon/libaxon_pjrt.so'))
    except Exception:
        pass


def _perm_row(v):
    """Node id -> permuted table row: (dest-half, core, window', lane)."""
    c = v // SLICE
    w = (v % SLICE) // P
    p = v % P
    dh = (w >= WH).astype(np.int64)
    wp = w - WH * dh
    return dh * HALFTAB + c * HROWS + wp * P + p


def _prep(x, edge_index, batch, dinv):
    """Host-side shard prep. Returns per-core arrays + static schedule."""
    row = np.asarray(edge_index[0], dtype=np.int64)
    col = np.asarray(edge_index[1], dtype=np.int64)

    c_of = col // SLICE
    w_of = (col % SLICE) // P
    d_rel = (col % P).astype(np.float16)
    dh_of = (w_of >= WH).astype(np.int64)      # dest half of the edge
    src_row = _perm_row(row)
    q_of = src_row // QSIZE
    src_rel = (src_row % QSIZE).astype(np.int16)

    # group (dh, q, w) per core; SPMD-uniform tile counts = max over cores
    # slot segments ordered (dh, q, w)
    gkey = (dh_of * NQ + q_of) * WPC + w_of    # w encodes dh too; fine
    key_c = c_of * (2 * NQ * WPC) + gkey
    counts = np.bincount(key_c, minlength=NCORES * 2 * NQ * WPC)
    counts = counts.reshape(NCORES, 2 * NQ, WPC)
    gmax = counts.max(axis=0)                  # [2*NQ, WPC]
    T = -(-gmax // P)                          # tiles per (dh*NQ+q, w)

    tiles_flat = T.reshape(-1)
    tile_base = np.zeros(2 * NQ * WPC, np.int64)
    np.cumsum(tiles_flat[:-1], out=tile_base[1:])
    n_tiles = int(tiles_flat.sum())
    n_slots = n_tiles * P

    # per-(dh, q) runs: gather calls and window spans
    runs = {}
    for dh in range(2):
        for q in range(NQ):
            gi = dh * NQ + q
            wlist = range(WH) if dh == 0 else range(WH, WPC)
            wins = [(w, int(tile_base[gi * WPC + w]), int(T[gi, w]))
                    for w in wlist if T[gi, w] > 0]
            if wins:
                rt0 = wins[0][1]
                rt1 = wins[-1][1] + wins[-1][2]
            else:
                rt0 = rt1 = 0
            calls = []
            t = rt0
            while t < rt1:
                nt = min(CHT, rt1 - t)
                calls.append((t, nt))
                t += nt
            runs[(dh, q)] = dict(calls=calls, wins=wins)

    first_q = np.full(WPC, -1, np.int64)
    for q in range(NQ - 1, -1, -1):
        for dh in range(2):
            gi = dh * NQ + q
            first_q[np.flatnonzero(T[gi] > 0)] = q

    # per-edge slot assignment (sorted by core, dh, quarter, window, src)
    order = np.lexsort((src_row, w_of, q_of, dh_of, c_of))
    k = key_c[order]
    change = np.empty(E, bool)
    change[0] = True
    change[1:] = k[1:] != k[:-1]
    starts = np.flatnonzero(change)
    grp = np.cumsum(change) - 1
    rank = np.arange(E) - starts[grp]
    slot = tile_base[gkey[order]] * P + rank

    srcrel_slots = np.zeros((NCORES, n_slots), np.int16)
    dcol_slots = np.full((NCORES, n_slots), SENTINEL, np.float16)
    cs = c_of[order]
    srcrel_slots[cs, slot] = src_rel[order]
    dcol_slots[cs, slot] = d_rel[order]

    idx16 = np.empty((NCORES, P, n_slots // 16), np.int16)
    dcol = np.empty((NCORES, P, n_tiles), np.float16)
    for c in range(NCORES):
        idx16[c] = np.tile(srcrel_slots[c].reshape(n_slots // 16, 16).T, (8, 1))
        dcol[c] = dcol_slots[c].reshape(n_tiles, P).T

    # node-side arrays
    xpad = np.zeros((NPAD, IN_DIM), np.float32)
    xpad[:N] = x
    dinv_pad = np.zeros(NPAD, np.float32)
    dinv_pad[:N] = dinv
    batch_pad = np.full(NPAD, SENTINEL, np.float32)
    batch_pad[:N] = np.asarray(batch, np.float32)

    # xdT in PERMUTED column order: phase-0 chunk c' covers permuted rows
    # [c'*SLICE, (c'+1)*SLICE) and writes them contiguously.
    inv = np.empty(NPAD, np.int64)
    inv[_perm_row(np.arange(NPAD))] = np.arange(NPAD)
    xdT = (xpad * dinv_pad[:, None])[inv].T.astype(np.float16)   # [64, NPAD]

    xTr = np.empty((NCORES, IN_DIM, SLICE), np.float16)
    dinvR = np.empty((NCORES, IN_DIM, SLICE), np.float16)
    dinv_cols = np.empty((NCORES, P, WPC), np.float32)
    batch_cols = np.empty((NCORES, P, WPC), np.float16)
    for c in range(NCORES):
        sl = slice(c * SLICE, (c + 1) * SLICE)
        xTr[c] = xpad[sl].T
        dinvR[c] = np.tile(dinv_pad[sl].astype(np.float16)[None, :], (IN_DIM, 1))
        dinv_cols[c] = dinv_pad[sl].reshape(WPC, P).T
        batch_cols[c] = batch_pad[sl].reshape(WPC, P).T.astype(np.float16)

    sched = dict(n_slots=n_slots, n_tiles=n_tiles, runs=runs,
                 first_q=first_q.tolist())
    data = dict(idx16=idx16, dcol=dcol, xdT=xdT, xTr=xTr, dinvR=dinvR,
                dinv_cols=dinv_cols, batch_cols=batch_cols)
    return sched, data


def _build(sched):
    """Build the SPMD Bass program (same for all cores)."""
    from contextlib import ExitStack

    n_tiles = sched['n_tiles']
    n_slots = sched['n_slots']
    runs = sched['runs']
    first_q = sched['first_q']

    nc = bacc.Bacc("TRN2", target_bir_lowering=False, debug=False,
                   num_devices=NCORES, num_swdge_queues=4)

    # I/O
    xdT_d = nc.dram_tensor("xdT", [IN_DIM, NPAD], F16, kind="ExternalInput")
    xTr_d = nc.dram_tensor("xTr", [IN_DIM, SLICE], F16, kind="ExternalInput")
    dinvR_d = nc.dram_tensor("dinvR", [IN_DIM, SLICE], F16, kind="ExternalInput")
    dinv_d = nc.dram_tensor("dinv_cols", [P, WPC], F32, kind="ExternalInput")
    batch_d = nc.dram_tensor("batch_cols", [P, WPC], F16, kind="ExternalInput")
    idx_d = nc.dram_tensor("idx16", [P, n_slots // 16], I16, kind="ExternalInput")
    dcol_d = nc.dram_tensor("dcol", [P, n_tiles], F16, kind="ExternalInput")
    w1i_d = nc.dram_tensor("w1i", [IN_DIM, HID], F16, kind="ExternalInput")
    w1r_d = nc.dram_tensor("w1r", [IN_DIM, HID], F16, kind="ExternalInput")
    b1r_d = nc.dram_tensor("b1r", [P, HID], F32, kind="ExternalInput")
    w2i_d = nc.dram_tensor("w2i", [HID, HID], F32, kind="ExternalInput")
    w2r_d = nc.dram_tensor("w2r", [HID, HID], F32, kind="ExternalInput")
    b2r_d = nc.dram_tensor("b2r", [P, HID], F32, kind="ExternalInput")
    fcwb_d = nc.dram_tensor("fcwb", [HID + 1, OUT], F32, kind="ExternalInput")
    cntinv_d = nc.dram_tensor("cntinv", [NUM_GRAPHS, 1], F32, kind="ExternalInput")
    out_d = nc.dram_tensor("out", [NUM_GRAPHS, OUT], F32, kind="ExternalOutput")

    # constants embedded in NEFF
    iota_oh_np = np.tile(np.arange(P, dtype=np.float16)[None, :], (P, CHT))
    iota_oh_d = nc.inline_tensor(iota_oh_np, name="iota_oh")
    iota_pool_np = np.tile(np.arange(NUM_GRAPHS, dtype=np.float16)[None, :], (P, 1))
    iota_pool_d = nc.inline_tensor(iota_pool_np, name="iota_pool")
    ident_d = nc.inline_tensor(np.eye(P, dtype=np.float32), name="ident128")

    # internal DRAM: g1 per-quarter (local, replicated build); g2 per-half
    g1q = [nc.dram_tensor(f"g1q{q}", [QSIZE, P], F16) for q in range(NQ)]
    g2loc = [nc.dram_tensor(f"g2loc{h}", [HROWS, P], F16) for h in range(2)]
    g2tab = [nc.dram_tensor(f"g2tab{h}", [HALFTAB, P], F16, addr_space="Shared")
             for h in range(2)]
    pin_d = nc.dram_tensor("pooled_in", [NUM_GRAPHS, HID], F32)
    pout_d = nc.dram_tensor("pooled_out", [NUM_GRAPHS, HID], F32,
                            addr_space="Shared")

    RG = [list(range(NCORES))]
    Copy = mybir.ActivationFunctionType.Copy

    with tile.TileContext(nc) as tc:
        with ExitStack() as ctx:
            consts = ctx.enter_context(tc.tile_pool(name="consts", bufs=1))
            canvas = ctx.enter_context(tc.tile_pool(name="canvas", bufs=4))
            gpool = ctx.enter_context(tc.tile_pool(name="gbuf", bufs=6))
            ohpool = ctx.enter_context(tc.tile_pool(name="oh", bufs=4))
            wk = ctx.enter_context(tc.tile_pool(name="wk", bufs=4))
            pseg = ctx.enter_context(tc.tile_pool(name="pseg", bufs=3, space="PSUM"))
            pbat = ctx.enter_context(tc.tile_pool(name="pbat", bufs=2, space="PSUM"))
            ptp = ctx.enter_context(tc.tile_pool(name="ptp", bufs=2, space="PSUM"))
            ppool = ctx.enter_context(tc.tile_pool(name="ppool", bufs=1, space="PSUM"))

            # ---- load constants ----
            iota_oh_sb = consts.tile([P, CHT * P], F16)
            nc.sync.dma_start(out=iota_oh_sb[:], in_=iota_oh_d[:, :])
            iota_pool_sb = consts.tile([P, NUM_GRAPHS], F16)
            nc.sync.dma_start(out=iota_pool_sb[:], in_=iota_pool_d[:, :])
            ident_sb = consts.tile([P, P], F32)
            nc.sync.dma_start(out=ident_sb[:], in_=ident_d[:, :])
            dinv_sb = consts.tile([P, WPC], F32)
            nc.sync.dma_start(out=dinv_sb[:], in_=dinv_d[:, :])
            batch_sb = consts.tile([P, WPC], F16)
            nc.sync.dma_start(out=batch_sb[:], in_=batch_d[:, :])
            idx_sb = consts.tile([P, n_slots // 16], I16)
            nc.sync.dma_start(out=idx_sb[:], in_=idx_d[:, :])
            dcol_sb = consts.tile([P, n_tiles], F16)
            nc.sync.dma_start(out=dcol_sb[:], in_=dcol_d[:, :])
            w1i_sb = consts.tile([IN_DIM, HID], F16)
            nc.sync.dma_start(out=w1i_sb[:], in_=w1i_d[:, :])
            w1r_sb = consts.tile([IN_DIM, HID], F16)
            nc.sync.dma_start(out=w1r_sb[:], in_=w1r_d[:, :])
            b1r_sb = consts.tile([P, HID], F32)
            nc.sync.dma_start(out=b1r_sb[:], in_=b1r_d[:, :])
            w2i_sb = consts.tile([HID, HID], F32)
            nc.sync.dma_start(out=w2i_sb[:], in_=w2i_d[:, :])
            w2r_sb = consts.tile([HID, HID], F32)
            nc.sync.dma_start(out=w2r_sb[:], in_=w2r_d[:, :])
            b2r_sb = consts.tile([P, HID], F32)
            nc.sync.dma_start(out=b2r_sb[:], in_=b2r_d[:, :])
            fcwb_sb = consts.tile([HID + 1, OUT], F32)
            nc.sync.dma_start(out=fcwb_sb[:], in_=fcwb_d[:, :])
            cntinv_sb = consts.tile([NUM_GRAPHS, 1], F32)
            nc.sync.dma_start(out=cntinv_sb[:], in_=cntinv_d[:, :])

            CSLOT_F32 = [P, WPC * HID]      # 25088B canvas slot, f32 view
            CSLOT_F16 = [P, WPC * P]        # same bytes, f16 view
            XSLOT = [IN_DIM, SLICE]         # same bytes, f16 on 64 partitions

            def canvas_tile(shape, dtype, name):
                return canvas.tile(shape, dtype, tag="big", name=name)

            # ---- phase 0: g1 table (permuted row order), replicated ----
            for cp in range(NCORES):        # permuted chunk = rows cp*SLICE..
                xTc = canvas_tile(XSLOT, F16, f"xTc{cp}")
                nc.sync.dma_start(out=xTc[:],
                                  in_=xdT_d[:, cp * SLICE:(cp + 1) * SLICE])
                stg = canvas_tile(CSLOT_F16, F16, f"stg{cp}")
                stg3 = stg[:].rearrange("p (w d) -> p w d", d=P)
                for b in range(0, WPC, 8):
                    nb = min(8, WPC - b)
                    bat = pbat.tile([P, 8 * HID], F32, tag="bat", name="p0bat")
                    bat3 = bat[:].rearrange("p (k d) -> p k d", d=HID)
                    for k in range(nb):
                        w = b + k
                        nc.tensor.matmul(
                            bat3[:, k, :],
                            lhsT=xTc[:, w * P:(w + 1) * P],
                            rhs=w1i_sb[:], start=True, stop=True)
                    nc.scalar.activation(stg3[:, b:b + nb, 0:HID],
                                         bat3[:, 0:nb, :], Copy)
                q, hh = cp // 2, cp % 2
                nc.sync.dma_start(
                    out=g1q[q][hh * SLICE:(hh + 1) * SLICE, :]
                        .rearrange("(w p) d -> p w d", p=P),
                    in_=stg3)

            qno = [0]

            def run_propagate(gsrc_ap, dh, q, accum3):
                """Gathers + one-hot matmuls + window accumulation for a run."""
                rr = runs[(dh, q)]
                win_of = {}
                for (w, t0, nt) in rr['wins']:
                    for t in range(t0, t0 + nt):
                        win_of[t] = (w, t == t0, t == t0 + nt - 1)
                seg = None
                for (t0, nt) in rr['calls']:
                    gbuf = gpool.tile([P, CHT * P], F16, tag="gb", name="gbuf")
                    gb3 = gbuf[:].rearrange("p (s d) -> p s d", d=P)
                    nc.gpsimd.dma_gather(
                        out_ap=gb3[:, 0:nt, :],
                        in_ap=gsrc_ap,
                        idxs_ap=idx_sb[:, t0 * 8:(t0 + nt) * 8],
                        num_idxs=nt * P, num_idxs_reg=nt * P, elem_size=P,
                        single_packet=False, queue_num=qno[0] % 4)
                    qno[0] += 1
                    oh = ohpool.tile([P, CHT * P], F16, tag="oh", name="oh")
                    oh3 = oh[:].rearrange("p (k d) -> p k d", d=P)
                    nc.vector.tensor_tensor(
                        oh3[:, 0:nt, :],
                        iota_oh_sb[:].rearrange("p (k d) -> p k d", d=P)[:, 0:nt, :],
                        dcol_sb[:, t0:t0 + nt].unsqueeze(2).to_broadcast([P, nt, P]),
                        mybir.AluOpType.is_equal)
                    for tt in range(nt):
                        t = t0 + tt
                        w, fst, lst = win_of[t]
                        if fst:
                            seg = pseg.tile([P, HID], F32, tag="seg", name="seg")
                        nc.tensor.matmul(
                            seg[:], lhsT=oh3[:, tt, :],
                            rhs=gb3[:, tt, 0:HID], start=fst, stop=lst)
                        if lst:
                            if first_q[w] == q:
                                nc.vector.tensor_copy(accum3[:, w, :], seg[:])
                            else:
                                nc.vector.tensor_add(
                                    accum3[:, w, :], accum3[:, w, :], seg[:])

            def root_build(rootbuf, xT_sb, w_sb, br_sb, w0, w1):
                r3 = rootbuf[:].rearrange("p (w d) -> p w d", d=HID)
                for b in range(w0, w1, 8):
                    nb = min(8, w1 - b)
                    bat = pbat.tile([P, 8 * HID], F32, tag="bat", name="rbat")
                    bat3 = bat[:].rearrange("p (k d) -> p k d", d=HID)
                    for k in range(nb):
                        w = b + k
                        nc.tensor.matmul(
                            bat3[:, k, :],
                            lhsT=xT_sb[:, w * P:(w + 1) * P],
                            rhs=w_sb[:], start=True, stop=True)
                    nc.vector.tensor_add(
                        r3[:, b:b + nb, :], bat3[:, 0:nb, :],
                        br_sb[:].unsqueeze(1).to_broadcast([P, nb, HID]))

            def fuse_h(hbuf, accum, rootbuf, w0, w1):
                """h[w0:w1] = relu(accum * dinv_dest + root) on half slices."""
                nw = w1 - w0
                h3 = hbuf[:].rearrange("p (w d) -> p w d", d=HID)[:, w0:w1, :]
                a3 = accum[:].rearrange("p (w d) -> p w d", d=HID)[:, w0:w1, :]
                nc.vector.tensor_tensor(
                    h3, a3,
                    dinv_sb[:, w0:w1].unsqueeze(2).to_broadcast([P, nw, HID]),
                    mybir.AluOpType.mult)
                fl = slice(w0 * HID, w1 * HID)
                nc.vector.tensor_add(hbuf[:, fl], hbuf[:, fl], rootbuf[:, fl])
                nc.vector.tensor_scalar_max(hbuf[:, fl], hbuf[:, fl], 0.0)

            # ---- canvas buffers (ring slot order matters; see tags) ----
            accum1 = canvas_tile(CSLOT_F32, F32, "accum1")
            accum13 = accum1[:].rearrange("p (w d) -> p w d", d=HID)

            # ---- layer 1 propagate, dest-half A then B ----
            for q in range(NQ):
                run_propagate(g1q[q][:, :], 0, q, accum13)

            # asm buffers: single tiles, written/read in per-half slices
            root1 = canvas_tile(CSLOT_F32, F32, "root1")
            h1 = canvas_tile(CSLOT_F32, F32, "h1")
            root2 = canvas_tile(CSLOT_F32, F32, "root2")
            r23 = root2[:].rearrange("p (w d) -> p w d", d=HID)
            h13 = h1[:].rearrange("p (w d) -> p w d", d=HID)

            def assemble1_pieces(hh):
                """Emit-closures for half hh's assembly, interleavable."""
                w0, w1 = (0, WH) if hh == 0 else (WH, WPC)

                def roots():
                    xTrH = wk.tile([IN_DIM, WH * P], F16, tag="xTr", bufs=1,
                                   name="xTrH")
                    nc.sync.dma_start(out=xTrH[:], in_=xTr_d[:, w0 * P:w1 * P])
                    r13 = root1[:].rearrange("p (w d) -> p w d", d=HID)
                    for b in range(w0, w1, 8):
                        nb = min(8, w1 - b)
                        bat = pbat.tile([P, 8 * HID], F32, tag="bat",
                                        name="rbat")
                        bat3 = bat[:].rearrange("p (k d) -> p k d", d=HID)
                        for k in range(nb):
                            w = b + k
                            nc.tensor.matmul(
                                bat3[:, k, :],
                                lhsT=xTrH[:, (w - w0) * P:(w - w0 + 1) * P],
                                rhs=w1r_sb[:], start=True, stop=True)
                        nc.vector.tensor_add(
                            r13[:, b:b + nb, :], bat3[:, 0:nb, :],
                            b1r_sb[:].unsqueeze(1).to_broadcast([P, nb, HID]))

                def act():
                    fuse_h(h1, accum1, root1, w0, w1)
                    dvH = wk.tile([IN_DIM, WH * P], F16, tag="dvR", bufs=1,
                                  name="dvH")
                    nc.sync.dma_start(out=dvH[:], in_=dinvR_d[:, w0 * P:w1 * P])
                    assemble1_pieces.dvH = dvH

                def g2half(sub):
                    dvH = assemble1_pieces.dvH
                    lo = w0 + sub * 24
                    hi = min(w1, lo + 24) if sub == 0 else w1
                    for b4 in range(lo, hi, 4):
                        n4 = min(4, hi - b4)
                        tpA = ptp.tile([IN_DIM, 4 * P], F32, tag="tp",
                                       name="tpA")
                        for k in range(n4):
                            w = b4 + k
                            nc.tensor.transpose(tpA[:, k * P:(k + 1) * P],
                                                h13[:, w, :], ident_sb[:])
                        h1T4 = wk.tile([IN_DIM, 4 * P], F32, tag="h1T", bufs=2,
                                       name="h1T4")
                        nc.scalar.activation(h1T4[:], tpA[:], Copy)
                        cs = slice((b4 - w0) * P, (b4 - w0 + n4) * P)
                        h1sT4 = wk.tile([IN_DIM, 4 * P], F32, tag="h1sT",
                                        bufs=2, name="h1sT4")
                        nc.vector.tensor_tensor(
                            h1sT4[:, 0:n4 * P], tpA[:, 0:n4 * P], dvH[:, cs],
                            mybir.AluOpType.mult)
                        batG = pbat.tile([P, 8 * HID], F32, tag="bat",
                                         name="batG")
                        batG3 = batG[:].rearrange("p (k d) -> p k d", d=HID)
                        batR = pbat.tile([P, 8 * HID], F32, tag="bat",
                                         name="batR")
                        batR3 = batR[:].rearrange("p (k d) -> p k d", d=HID)
                        for k in range(n4):
                            nc.tensor.matmul(batG3[:, k, :],
                                             lhsT=h1sT4[:, k * P:(k + 1) * P],
                                             rhs=w2i_sb[:], start=True,
                                             stop=True)
                            nc.tensor.matmul(batR3[:, k, :],
                                             lhsT=h1T4[:, k * P:(k + 1) * P],
                                             rhs=w2r_sb[:], start=True,
                                             stop=True)
                        sg4 = wk.tile([P, 4 * P], F16, tag="sg4", bufs=2,
                                      name="sg4")
                        sg43 = sg4[:].rearrange("p (w d) -> p w d", d=P)
                        nc.scalar.activation(sg43[:, 0:n4, 0:HID],
                                             batG3[:, 0:n4, :], Copy)
                        nc.vector.tensor_add(
                            r23[:, b4:b4 + n4, :], batR3[:, 0:n4, :],
                            b2r_sb[:].unsqueeze(1).to_broadcast([P, n4, HID]))
                        nc.sync.dma_start(
                            out=g2loc[hh][(b4 - w0) * P:(b4 - w0 + n4) * P, :]
                                .rearrange("(w p) d -> p w d", p=P),
                            in_=sg43[:, 0:n4, :])

                return [roots, act, lambda: g2half(0), lambda: g2half(1)]

            # half-B propagate with asm-A pieces interleaved between runs
            pieces = assemble1_pieces(0)
            for q in range(NQ):
                run_propagate(g1q[q][:, :], 1, q, accum13)
                pieces[q]()

            # AllGather chunk A (emitted before half-B gathers would... the
            # Pool queue order here is: L1 gathers, AG-A, L2 q0/q1, AG-B, rest)
            nc.gpsimd.collective_compute(
                "AllGather", mybir.AluOpType.bypass, replica_groups=RG,
                ins=[g2loc[0].ap().opt()], outs=[g2tab[0].ap().opt()])

            # half-B assembly (before accum2 reuses accum1's canvas slot)
            for piece in assemble1_pieces(1):
                piece()

            # ---- layer 2 propagate: src quarters 0,1 (table chunk A) ----
            accum2 = canvas_tile(CSLOT_F32, F32, "accum2")
            accum23 = accum2[:].rearrange("p (w d) -> p w d", d=HID)
            for q in (0, 1):
                src = g2tab[0][(q % 2) * QSIZE:(q % 2 + 1) * QSIZE, :]
                for dh in range(2):
                    run_propagate(src, dh, q, accum23)

            nc.gpsimd.collective_compute(
                "AllGather", mybir.AluOpType.bypass, replica_groups=RG,
                ins=[g2loc[1].ap().opt()], outs=[g2tab[1].ap().opt()])

            for q in (2, 3):
                src = g2tab[1][(q % 2) * QSIZE:(q % 2 + 1) * QSIZE, :]
                for dh in range(2):
                    run_propagate(src, dh, q, accum23)

            # ---- layer 2 assembly + pooling ----
            h2 = canvas_tile(CSLOT_F32, F32, "h2")
            fuse_h(h2, accum2, root2, 0, WPC)
            ohB = canvas_tile(CSLOT_F32, F32, "ohB")
            ohB3 = ohB[:].rearrange("p (w d) -> p w d", d=HID)
            nc.vector.tensor_tensor(
                ohB3,
                iota_pool_sb[:].unsqueeze(1).to_broadcast([P, WPC, NUM_GRAPHS]),
                batch_sb[:].unsqueeze(2).to_broadcast([P, WPC, NUM_GRAPHS]),
                mybir.AluOpType.is_equal)
            h23 = h2[:].rearrange("p (w d) -> p w d", d=HID)
            pool_ps = ppool.tile([NUM_GRAPHS, HID], F32)
            for w in range(WPC):
                nc.tensor.matmul(pool_ps[:], lhsT=ohB3[:, w, :],
                                 rhs=h23[:, w, :],
                                 start=(w == 0), stop=(w == WPC - 1))

            pooled_sb = wk.tile([NUM_GRAPHS, HID], F32, tag="fin", name="pooled")
            nc.vector.tensor_copy(pooled_sb[:], pool_ps[:])
            nc.sync.dma_start(out=pin_d[:, :], in_=pooled_sb[:])
            nc.gpsimd.collective_compute(
                "AllReduce", mybir.AluOpType.add, replica_groups=RG,
                ins=[pin_d.ap().opt()], outs=[pout_d.ap().opt()])
            pg = wk.tile([NUM_GRAPHS, HID], F32, tag="fin", name="pg")
            nc.sync.dma_start(out=pg[:], in_=pout_d[:, :])
            pm = wk.tile([NUM_GRAPHS, HID], F32, tag="fin", name="pm")
            nc.vector.tensor_scalar_mul(pm[:], pg[:], cntinv_sb[:])
            tpf = ptp.tile([IN_DIM, 4 * P], F32, tag="tp", name="tpf")
            nc.tensor.transpose(tpf[0:HID, 0:NUM_GRAPHS], pm[:],
                                ident_sb[0:NUM_GRAPHS, 0:NUM_GRAPHS])
            lhs_fc = wk.tile([HID + 1, NUM_GRAPHS], F32, tag="fin", name="lhsfc")
            nc.vector.tensor_copy(lhs_fc[0:HID, :], tpf[0:HID, 0:NUM_GRAPHS])
            nc.vector.memset(lhs_fc[HID:HID + 1, :], 1.0)
            out_ps = pbat.tile([P, 8 * HID], F32, tag="bat", name="outps")
            nc.tensor.matmul(out_ps[0:NUM_GRAPHS, 0:OUT], lhsT=lhs_fc[:],
                             rhs=fcwb_sb[:], start=True, stop=True)
            osb = wk.tile([NUM_GRAPHS, OUT], F32, tag="fin", name="osb")
            nc.vector.tensor_copy(osb[:], out_ps[0:NUM_GRAPHS, 0:OUT])
            nc.sync.dma_start(out=out_d[:, :], in_=osb[:])

    nc.compile()
    return nc


def kernel(x, edge_index, batch, w1_init, w1_root, b1, w2_init, w2_root, b2,
           fc_w, fc_b):
    x = np.asarray(x, np.float32)
    edge_index = np.asarray(edge_index)
    batch = np.asarray(batch)

    col = edge_index[1].astype(np.int64)
    deg = np.bincount(col, minlength=N).astype(np.float32)
    dinv = np.where(deg > 0, 1.0 / np.sqrt(np.maximum(deg, 1.0)), 0.0).astype(np.float32)

    sched, data = _prep(x, edge_index, batch, dinv)

    key = (sched['n_slots'], sched['n_tiles'])
    if key not in _cache:
        _cache[key] = _build(sched)
    nc = _cache[key]

    cnt = np.bincount(batch.astype(np.int64), minlength=NUM_GRAPHS).astype(np.float32)
    cntinv = (1.0 / np.maximum(cnt, 1.0)).reshape(NUM_GRAPHS, 1)
    fcwb = np.vstack([np.asarray(fc_w, np.float32),
                      np.asarray(fc_b, np.float32)[None, :]])
    b1r = np.tile(np.asarray(b1, np.float32)[None, :], (P, 1))
    b2r = np.tile(np.asarray(b2, np.float32)[None, :], (P, 1))

    in_maps = []
    for c in range(NCORES):
        in_maps.append({
            "xdT": data['xdT'],
            "xTr": data['xTr'][c],
            "dinvR": data['dinvR'][c],
            "dinv_cols": data['dinv_cols'][c],
            "batch_cols": data['batch_cols'][c],
            "idx16": data['idx16'][c],
            "dcol": data['dcol'][c],
            "w1i": np.asarray(w1_init, np.float16),
            "w1r": np.asarray(w1_root, np.float16),
            "b1r": b1r,
            "w2i": np.asarray(w2_init, np.float32),
            "w2r": np.asarray(w2_root, np.float32),
            "b2r": b2r,
            "fcwb": fcwb,
            "cntinv": cntinv,
        })

    trace = os.environ.get("GNN_TRACE", "0") == "1"
    kw = {}
    if trace:
        _install_trace_hook()
        kw = dict(trace=True, tmpdir=os.environ.get("GNN_TRACEDIR") or None)
    res = run_bass_kernel_spmd(nc, in_maps, core_ids=list(range(NCORES)), **kw)
    kernel.last_exec_time_ns = res.exec_time_ns
    return np.asarray(res.results[0]["out"], np.float32)
